# revision 19
# baseline (speedup 1.0000x reference)
"""Trainium2 Bass kernel for nn_BDHLayer (sparse attention / BDH layer).

Sharding: 16 heads across 8 cores (2 heads per core, tensor parallel).
Decoder partial sums are combined with an on-chip ReduceScatter (bf16);
each core then applies the final layernorm+residual+rmsnorm to its T/8
slice.

All matmuls run in bf16 (fp32 PSUM accumulation). Host pre-transposes
weights/activations so every contraction dim lands on SBUF partitions.

v3 numerics: the middle layernorm is applied as a post-GEMM correction
  (as v2), but its stats (mean and sum-of-squares over the partition
  axis) are computed with DVE pairwise trees + a single ones-matmul
  each, instead of 16 ones-matmuls per t-block on the PE.

v3 scheduling vs v2:
- All big SBUF tiles live in pools shared across the two heads (same
  tag, bufs=1/2): head h+1's weight/x loads depend only on head h's
  last reader of the same buffer, so they stream in one phase early
  instead of waiting for h's gating scratch to free.
- A tiny dummy ReduceScatter early in h0's enc warms the collective
  path, so the first real RS runs at full speed.
- The decoder is split into 6 chunks (4,4,4,2,1,1 t-tiles); each chunk's
  ReduceScatter hides behind the next chunk's matmuls and the final
  exposed chunk is only 128 rows. Final norms are deferred two chunks
  so their RS-waits never head-of-line block the scalar queue.
"""

import sys

sys.path.insert(0, '/opt/trn_rl_repo')

import numpy as np
import ml_dtypes

import concourse.bass as bass
import concourse.bacc as bacc
import concourse.mybir as mybir
from concourse import tile
from concourse import bass_utils
from concourse import bass_isa

BF = ml_dtypes.bfloat16
FP32 = np.float32

B, T, D = 1, 2048, 1024
NH = 16
N = 1024            # neurons per head
CS = 256            # rotary chunk size
BASE = 2.0 ** 16
SCALE_BASE = 512.0
LN_EPS = 1e-5
RMS_EPS = 1e-6

NCORES = 8
HPC = NH // NCORES  # heads per core = 2
TS = T // NCORES    # output rows per core = 256

NT = N // 128       # 8 n-tiles per head
DT = D // 128       # 8 d-tiles
TT = T // 128       # 16 t-tiles
TB = T // 512       # 4 t-blocks
DB = D // 512       # 2 d-blocks

# decoder / reduce-scatter chunks: (first t-tile, n t-tiles)
RS_CHUNKS = ((12, 4), (0, 4), (4, 4), (8, 3), (11, 1))
GATE_ORDER = (3, 0, 1, 2)

dt = mybir.dt
Alu = mybir.AluOpType
Act = mybir.ActivationFunctionType


# ---------------------------------------------------------------- host tables

def _rope_tables():
    idx = np.arange(0, CS, 2, dtype=np.float64)
    inv_freq = 1.0 / (BASE ** (idx / CS))
    t = np.arange(T, dtype=np.float64)
    freqs = t[:, None] * inv_freq[None, :]              # (T, 128)
    scale_vec = (idx + 0.4 * CS) / (1.4 * CS)
    power = (t - T // 2) / SCALE_BASE
    scale = scale_vec[None, :] ** power[:, None]        # (T, 128)
    cos = (np.cos(freqs) * scale).astype(np.float32)
    sin = (np.sin(freqs) * scale).astype(np.float32)
    # transpose to (128, T): row = pair index within chunk, col = t
    return np.ascontiguousarray(cos.T), np.ascontiguousarray(sin.T)


def _masks():
    # scoresT tile layout: [u_p (128), t_f (512)]; diagonal block j keeps
    # strictly-causal u < t, i.e. 128*j + u_p < t_f.
    m = np.zeros((4, 128, 512), dtype=np.float32)
    up = np.arange(128)[:, None]
    tf = np.arange(512)[None, :]
    for j in range(4):
        m[j] = (128 * j + up < tf).astype(np.float32)
    return m


# ------------------------------------------------------------------- builder

def _emit(nc, tc, tens):
    x_bf, xT_bf, xs_f32 = tens['x_bf'], tens['xT_bf'], tens['xs_f32']
    wencT, wencvT, wdecT = tens['wencT'], tens['wencvT'], tens['wdecT']
    wsumT, cosT_d, sinT_d, masks_d = (tens['wsumT'], tens['cosT'],
                                      tens['sinT'], tens['masks'])
    out_d, xy_d = tens['out'], tens['xy_d']
    bounce_in, bounce_out = tens['bounce_in'], tens['bounce_out']
    warm_in, warm_out = tens['warm_in'], tens['warm_out']

    f32, bf16 = dt.float32, dt.bfloat16

    from contextlib import ExitStack
    with ExitStack() as ctx:
        p_const = ctx.enter_context(
            tc.tile_pool(name="const", bufs=1, side="left"))
        p_wv = ctx.enter_context(
            tc.tile_pool(name="wv", bufs=1, side="left"))
        p_psum = ctx.enter_context(
            tc.tile_pool(name="psum", bufs=6, space="PSUM"))
        p_psum_v = ctx.enter_context(
            tc.tile_pool(name="psumv", bufs=2, space="PSUM"))
        p_head = ctx.enter_context(
            tc.tile_pool(name="head", bufs=1, side="right"))

        cos_sb = p_const.tile([128, T], bf16, tag="cos")
        sin_sb = p_const.tile([128, T], bf16, tag="sin")
        mask_sb = p_const.tile([128, 4 * 512], bf16, tag="masks")
        wsum_sb = p_const.tile([128, HPC * NT], f32, tag="wsum")
        ones_bf = p_const.tile([128, 1], bf16, tag="ones_bf")
        nc.vector.memset(ones_bf[:], 1.0)

        def head_tiles():
            qsq = p_head.tile([128, NT * T], bf16, tag="qsq")
            # per-t-block ykv tiles: dependency tracking is per tile, so
            # a single tile would stall t-block 0's z matmuls on
            # t-block 3's psum drains
            ykv = [p_head.tile([128, DT * 512], bf16, tag=f"ykv{i}",
                               name=f"ykv{i}")
                   for i in range(TB)]
            # per-t-block stat tiles: a shared [128, T] tile would make
            # t-block 0's gating wait on t-block 3's broadcast
            mu_b = [p_head.tile([128, 512], bf16, tag=f"mu_b{i}",
                                name=f"mu_b{i}")
                    for i in range(TB)]
            r2_b = [p_head.tile([128, 512], bf16, tag=f"r2_b{i}",
                                name=f"r2_b{i}")
                    for i in range(TB)]
            return qsq, ykv, mu_b, r2_b

        def emit_enc_compute(h, ectx_pools, qsq, qtr, wenc_sb, xf_t,
                             xf_next, load_xf):
            p_scr, p_rt = ectx_pools
            for tb in range(TB):
                tsl = slice(tb * 512, (tb + 1) * 512)
                xfull = xf_t
                for nt in range(NT):
                    ps = p_psum.tile([128, 512], f32, tag="mm")
                    for dtt in range(DT):
                        nc.tensor.matmul(
                            ps[:],
                            wenc_sb[:, dtt * N + nt * 128:
                                    dtt * N + nt * 128 + 128],
                            xfull[:, dtt * 512:(dtt + 1) * 512],
                            start=(dtt == 0), stop=(dtt == DT - 1))
                    relu_t = p_scr.tile([128, 512], f32, tag="relu")
                    nc.scalar.activation(relu_t[:], ps[:], Act.Relu)
                    nc.vector.tensor_mul(
                        qsq[:, nt * T + tb * 512:
                            nt * T + tb * 512 + 512],
                        relu_t[:], relu_t[:])
                    if nt % 2 == 1:
                        c = nt // 2
                        a = qsq[:, (2 * c) * T + tb * 512:
                                (2 * c) * T + (tb + 1) * 512]
                        b = qsq[:, (2 * c + 1) * T + tb * 512:
                                (2 * c + 1) * T + (tb + 1) * 512]
                        t1 = p_rt.tile([128, 512], bf16, tag="rt")
                        t2 = p_rt.tile([128, 512], bf16, tag="rt")
                        nc.vector.tensor_mul(t1[:], a, cos_sb[:, tsl])
                        nc.vector.tensor_mul(t2[:], b, sin_sb[:, tsl])
                        nc.vector.tensor_sub(
                            qtr[:, (2 * c) * T + tb * 512:
                                (2 * c) * T + (tb + 1) * 512],
                            t1[:], t2[:])
                        t3 = p_rt.tile([128, 512], bf16, tag="rt")
                        t4 = p_rt.tile([128, 512], bf16, tag="rt")
                        nc.vector.tensor_mul(t3[:], b, cos_sb[:, tsl])
                        nc.vector.tensor_mul(t4[:], a, sin_sb[:, tsl])
                        nc.vector.tensor_add(
                            qtr[:, (2 * c + 1) * T + tb * 512:
                                (2 * c + 1) * T + (tb + 1) * 512],
                            t3[:], t4[:])
                xf_t = xf_next
                if tb < TB - 2:
                    xf_next = load_xf(tb + 2)

        def emit_scores_ykv(h, sctx_pools, qtr, x_sb, ykv, mu_b, r2_b,
                            defer_last=False):
            p_sct, p_st, p_row = sctx_pools
            deferred = []
            for tb in range(TB):
                ub_max = 4 * tb + 4
                sct = p_sct.tile([128, 16 * 512], bf16, tag="sct")
                for ub in range(ub_max):
                    j = ub - 4 * tb
                    off = 128 * j if j > 0 else 0
                    w = 512 - off
                    ps = p_psum.tile([128, 512], f32, tag="mm")
                    for nt in range(NT):
                        nc.tensor.matmul(
                            ps[:, :w],
                            qtr[:, nt * T + ub * 128:
                                nt * T + ub * 128 + 128],
                            qtr[:, nt * T + tb * 512 + off:
                                nt * T + (tb + 1) * 512],
                            start=(nt == 0), stop=(nt == NT - 1))
                    base = ub * 512
                    if j >= 0:
                        nc.vector.tensor_mul(
                            sct[:, base + off:base + 512],
                            ps[:, :w],
                            mask_sb[:, j * 512 + off:(j + 1) * 512])
                    else:
                        nc.scalar.copy(sct[:, base:base + 512], ps[:])

                # yKV with incremental LN stats: running elementwise
                # sums of ykv and ykv^2 ride the DVE behind each psum
                # drain, so only a tiny tail remains after the last dtt.
                mA = p_st.tile([128, 512], bf16, tag="ts")
                sA = p_st.tile([128, 512], bf16, tag="ts")
                q0 = p_st.tile([128, 512], bf16, tag="ts")
                for dtt in range(DT):
                    ps2 = p_psum.tile([128, 512], f32, tag="mm")
                    for ub in range(ub_max):
                        j = ub - 4 * tb
                        off = 128 * j if j > 0 else 0
                        nc.tensor.matmul(
                            ps2[:, off:],
                            x_sb[:, ub * D + dtt * 128:
                                 ub * D + dtt * 128 + 128],
                            sct[:, ub * 512 + off:(ub + 1) * 512],
                            start=(ub == 0), stop=(ub == ub_max - 1))
                    ysl = ykv[tb][:, dtt * 512:(dtt + 1) * 512]
                    nc.scalar.copy(ysl, ps2[:])
                    if dtt == 0:
                        nc.vector.tensor_mul(sA[:], ysl, ysl)
                    else:
                        nc.vector.tensor_mul(q0[:], ysl, ysl)
                        nc.vector.tensor_add(sA[:], sA[:], q0[:])
                    if dtt == 1:
                        prev = ykv[tb][:, 0:512]
                        nc.vector.tensor_add(mA[:], prev, ysl)
                    elif dtt > 1:
                        nc.vector.tensor_add(mA[:], mA[:], ysl)
                mean_ps = p_psum_v.tile([1, 512], f32, tag="st")
                nc.tensor.matmul(mean_ps[:], ones_bf[:], mA[:],
                                 start=True, stop=True)
                ssq_ps = p_psum_v.tile([1, 512], f32, tag="st")
                nc.tensor.matmul(ssq_ps[:], ones_bf[:], sA[:],
                                 start=True, stop=True)
                if defer_last and tb == TB - 1:
                    # finalize after the next pool boundary so its tail
                    # (3us DVE reciprocal) is off the boundary barrier
                    deferred.append((mean_ps, ssq_ps, tb))
                    continue
                mu_r = p_row.tile([1, 512], bf16, tag="mu_r")
                ssq_r = p_row.tile([1, 512], f32, tag="ssq_r")
                musq_r = p_row.tile([1, 512], bf16, tag="musq_r")
                r2_r = p_row.tile([1, 512], bf16, tag="r2_r")
                nc.scalar.mul(mu_r[:], mean_ps[:], 1.0 / D)
                nc.scalar.mul(ssq_r[:], ssq_ps[:], 1.0 / D)
                nc.vector.tensor_mul(musq_r[:], mu_r[:], mu_r[:])
                nc.vector.tensor_sub(ssq_r[:], ssq_r[:], musq_r[:])
                nc.vector.tensor_scalar_add(ssq_r[:], ssq_r[:], LN_EPS)
                nc.vector.reciprocal(ssq_r[:], ssq_r[:])
                nc.vector.tensor_copy(r2_r[:], ssq_r[:])
                nc.gpsimd.partition_broadcast(
                    mu_b[tb][:, :], mu_r[:], channels=128)
                nc.gpsimd.partition_broadcast(
                    r2_b[tb][:, :], r2_r[:], channels=128)
            return deferred

        def emit_z_gate(h, p_gate, qsq, ykv, mu_b, r2_b, wv_sb, tb,
                        xy1=None):
            mu_t, r2_t = mu_b[tb], r2_b[tb]
            for nt in range(NT):
                ps3 = p_psum.tile([128, 512], f32, tag="mm")
                for dtt in range(DT):
                    nc.tensor.matmul(
                        ps3[:],
                        wv_sb[:, dtt * N + nt * 128:
                              dtt * N + nt * 128 + 128],
                        ykv[tb][:, dtt * 512:(dtt + 1) * 512],
                        start=(dtt == 0), stop=(dtt == DT - 1))
                v_t = p_gate.tile([128, 512], f32, tag="v")
                nc.vector.scalar_tensor_tensor(
                    v_t[:], mu_t[:, :],
                    wsum_sb[:, h * NT + nt:h * NT + nt + 1],
                    ps3[:], op0=Alu.mult, op1=Alu.subtract)
                t1_t = p_gate.tile([128, 512], bf16, tag="t1")
                nc.scalar.activation(t1_t[:], v_t[:], Act.Relu,
                                     scale=-1.0)
                g_t = p_gate.tile([128, 512], bf16, tag="g")
                nc.scalar.square(g_t[:], t1_t[:])
                tmp_t = p_gate.tile([128, 512], bf16, tag="tmp")
                nc.vector.tensor_mul(
                    tmp_t[:], g_t[:],
                    qsq[:, nt * T + tb * 512:nt * T + (tb + 1) * 512])
                if xy1 is None:
                    nc.vector.tensor_mul(tmp_t[:], tmp_t[:],
                                         r2_t[:, :])
                    nc.gpsimd.dma_start(
                        xy_d[nt, :, tb * 512:(tb + 1) * 512], tmp_t[:])
                else:
                    nc.vector.tensor_mul(
                        xy1[:, nt * 512:(nt + 1) * 512],
                        tmp_t[:], r2_t[:, :])

        with ExitStack() as bctx:
            p_mid = bctx.enter_context(
                tc.tile_pool(name="mid", bufs=1, side="right"))
            p_enc = bctx.enter_context(
                tc.tile_pool(name="enc", bufs=1, side="right"))

            # =================== H0: tiles + enc loads ==================
            qsq0, ykv0, mu0, r20 = head_tiles()
            wv0 = p_wv.tile([128, DT * N], bf16, tag="wv")
            qtr0 = p_mid.tile([128, NT * T], bf16, tag="qtr")
            xsb0 = p_mid.tile([128, TT * D], bf16, tag="x")
            wenc0 = p_enc.tile([128, DT * N], bf16, tag="wenc")

            ectx0 = ExitStack()
            p_xf0 = ectx0.enter_context(
                tc.tile_pool(name="xf0", bufs=2, side="right"))
            p_scr0 = ectx0.enter_context(
                tc.tile_pool(name="scr0", bufs=3, side="right"))
            p_rt0 = ectx0.enter_context(
                tc.tile_pool(name="rt0", bufs=4, side="right"))

            def load_xf0(tb):
                t = p_xf0.tile([128, DT * 512], bf16, tag="xf")
                for dtt in range(DT):
                    eng = nc.gpsimd if (tb + dtt) % 2 == 0 else nc.sync
                    eng.dma_start(
                        t[:, dtt * 512:(dtt + 1) * 512],
                        xT_bf[dtt * 128:(dtt + 1) * 128,
                              tb * 512:(tb + 1) * 512])
                return t

            for dtt in range(DT):
                nc.sync.dma_start(
                    wenc0[:, dtt * N:dtt * N + 512],
                    wencT[0, dtt * 128:(dtt + 1) * 128, 0:512])
            xf_t0 = p_xf0.tile([128, DT * 512], bf16, tag="xf")
            for dtt in range(DT):
                nc.gpsimd.dma_start(
                    xf_t0[:, dtt * 512:(dtt + 1) * 512],
                    xT_bf[dtt * 128:(dtt + 1) * 128, 0:512])
            # warm up the collective path with a throwaway RS so the
            # first real one runs at full speed
            nc.gpsimd.collective_compute(
                "ReduceScatter", Alu.add,
                replica_groups=[list(range(NCORES))],
                ins=[warm_in[:, :].opt()],
                outs=[warm_out[:, :].opt()])
            for dtt in range(DT):
                nc.sync.dma_start(
                    wenc0[:, dtt * N + 512:(dtt + 1) * N],
                    wencT[0, dtt * 128:(dtt + 1) * 128, 512:N])
            nc.gpsimd.dma_start(cos_sb[:], cosT_d[:])
            nc.gpsimd.dma_start(sin_sb[:], sinT_d[:])
            xf_n0 = load_xf0(1)
            for j in range(4):
                nc.gpsimd.dma_start(
                    mask_sb[:, j * 512:(j + 1) * 512], masks_d[j, :, :])
            for hh in range(HPC):
                nc.gpsimd.dma_start(
                    wsum_sb[:, hh * NT:(hh + 1) * NT], wsumT[hh, :, :])

            # =================== H0: enc compute =======================
            emit_enc_compute(0, (p_scr0, p_rt0), qsq0, qtr0, wenc0,
                             xf_t0, xf_n0, load_xf0)
            for tt in range(TT):
                eng = nc.sync if tt % 2 == 0 else nc.gpsimd
                eng.dma_start(xsb0[:, tt * D:(tt + 1) * D],
                              x_bf[tt * 128:(tt + 1) * 128, :])
            ectx0.close()

            # =================== H0: scores + yKV ======================
            for dtt in range(DT):
                nc.gpsimd.dma_start(
                    wv0[:, dtt * N:(dtt + 1) * N],
                    wencvT[0, dtt * 128:(dtt + 1) * 128, :])
            sctx0 = ExitStack()
            p_sct0 = sctx0.enter_context(
                tc.tile_pool(name="sct0", bufs=1, side="right"))
            p_st0 = sctx0.enter_context(
                tc.tile_pool(name="st0", bufs=3, side="right"))
            p_row0 = sctx0.enter_context(
                tc.tile_pool(name="row0", bufs=1, side="right"))
            emit_scores_ykv(0, (p_sct0, p_st0, p_row0), qtr0, xsb0,
                            ykv0, mu0, r20)

            # ============ H1: tiles + hoisted enc loads ================
            # Emitted before h0's gating so these DMAs stream during the
            # scores/gating phases (sync queue only -- the scalar queue
            # must stay clear for h0's gating activations).
            qsq1, ykv1, mu1, r21 = head_tiles()
            qtr1 = p_mid.tile([128, NT * T], bf16, tag="qtr")
            xsb1 = p_mid.tile([128, TT * D], bf16, tag="x")
            wenc1 = p_enc.tile([128, DT * N], bf16, tag="wenc")
            for dtt in range(DT):
                nc.sync.dma_start(
                    wenc1[:, dtt * N:dtt * N + 512],
                    wencT[1, dtt * 128:(dtt + 1) * 128, 0:512])
            for dtt in range(DT):
                nc.sync.dma_start(
                    wenc1[:, dtt * N + 512:(dtt + 1) * N],
                    wencT[1, dtt * 128:(dtt + 1) * 128, 512:N])
            sctx0.close()
            ectx1 = ExitStack()
            p_xf1 = ectx1.enter_context(
                tc.tile_pool(name="xf1", bufs=2, side="right"))

            def load_xf1(tb, engs=(nc.sync, nc.sync)):
                t = p_xf1.tile([128, DT * 512], bf16, tag="xf")
                for dtt in range(DT):
                    eng = engs[(tb + dtt) % 2]
                    eng.dma_start(
                        t[:, dtt * 512:(dtt + 1) * 512],
                        xT_bf[dtt * 128:(dtt + 1) * 128,
                              tb * 512:(tb + 1) * 512])
                return t

            xf_t1 = load_xf1(0)
            xf_n1 = load_xf1(1)
            for tt in range(TT):
                nc.sync.dma_start(xsb1[:, tt * D:(tt + 1) * D],
                                  x_bf[tt * 128:(tt + 1) * 128, :])

            # =================== H0: z / gating ========================
            with ExitStack() as g0ctx:
                p_gate0 = g0ctx.enter_context(
                    tc.tile_pool(name="gate0", bufs=2, side="left"))
                for tb in range(TB):
                    emit_z_gate(0, p_gate0, qsq0, ykv0, mu0, r20, wv0,
                                tb)

            # =================== H1: enc compute =======================
            p_scr1 = ectx1.enter_context(
                tc.tile_pool(name="scr1", bufs=3, side="right"))
            p_rt1 = ectx1.enter_context(
                tc.tile_pool(name="rt1", bufs=4, side="right"))
            wv1 = p_wv.tile([128, DT * N], bf16, tag="wv")
            for dtt in range(DT):
                nc.gpsimd.dma_start(
                    wv1[:, dtt * N:(dtt + 1) * N],
                    wencvT[1, dtt * 128:(dtt + 1) * 128, :])
            emit_enc_compute(
                1, (p_scr1, p_rt1), qsq1, qtr1, wenc1, xf_t1, xf_n1,
                lambda tb: load_xf1(tb, engs=(nc.scalar, nc.sync)))
            ectx1.close()

            # =================== H1: scores + yKV ======================
            sctx1 = ExitStack()
            p_sct1 = sctx1.enter_context(
                tc.tile_pool(name="sct1", bufs=1, side="right"))
            p_st1 = sctx1.enter_context(
                tc.tile_pool(name="st1", bufs=3, side="right"))
            p_row1 = sctx1.enter_context(
                tc.tile_pool(name="row1", bufs=1, side="right"))
            deferred1 = emit_scores_ykv(
                1, (p_sct1, p_st1, p_row1), qtr1, xsb1,
                ykv1, mu1, r21, defer_last=True)
            sctx1.close()

        # ============ H1 GATING + DECODER + RS + FINAL NORMS ============
        with ExitStack() as gctx:
            p_wd = gctx.enter_context(
                tc.tile_pool(name="wd", bufs=1, side="right"))
            p_xy1 = gctx.enter_context(
                tc.tile_pool(name="xy1", bufs=2, side="right"))
            p_xy0 = gctx.enter_context(
                tc.tile_pool(name="xy0", bufs=2, side="right"))
            p_ym = gctx.enter_context(
                tc.tile_pool(name="ym", bufs=3, side="right"))
            p_fin = gctx.enter_context(
                tc.tile_pool(name="fin", bufs=1, side="right"))
            p_gate1 = gctx.enter_context(
                tc.tile_pool(name="gate1", bufs=3, side="left"))
            wd_sb = p_wd.tile([128, HPC * NT * D], bf16, tag="wd")
            for r in range(HPC * NT):
                eng = nc.sync if r % 2 == 1 else nc.gpsimd
                eng.dma_start(wd_sb[:, r * D:(r + 1) * D],
                              wdecT[r * 128:(r + 1) * 128, :])

            # deferred tb3 stat finalize (see emit_scores_ykv)
            for mean_ps, ssq_ps, tbd in deferred1:
                mu_r = p_gate1.tile([1, 512], bf16, tag="dmu", bufs=1)
                ssq_r = p_gate1.tile([1, 512], f32, tag="dssq", bufs=1)
                musq_r = p_gate1.tile([1, 512], bf16, tag="dmusq",
                                      bufs=1)
                r2_r = p_gate1.tile([1, 512], bf16, tag="dr2", bufs=1)
                nc.scalar.mul(mu_r[:], mean_ps[:], 1.0 / D)
                nc.scalar.mul(ssq_r[:], ssq_ps[:], 1.0 / D)
                nc.vector.tensor_mul(musq_r[:], mu_r[:], mu_r[:])
                nc.vector.tensor_sub(ssq_r[:], ssq_r[:], musq_r[:])
                nc.vector.tensor_scalar_add(ssq_r[:], ssq_r[:], LN_EPS)
                nc.vector.reciprocal(ssq_r[:], ssq_r[:])
                nc.vector.tensor_copy(r2_r[:], ssq_r[:])
                nc.gpsimd.partition_broadcast(
                    mu1[tbd][:, :], mu_r[:], channels=128)
                nc.gpsimd.partition_broadcast(
                    r21[tbd][:, :], r2_r[:], channels=128)

            def fetch_xy0(tb):
                t = p_xy0.tile([128, NT * 512], bf16, tag="xy0")
                for nt in range(NT):
                    nc.scalar.dma_start(
                        t[:, nt * 512:(nt + 1) * 512],
                        xy_d[nt, :, tb * 512:(tb + 1) * 512])
                return t
            xy0_t = fetch_xy0(GATE_ORDER[0])
            xy0_next = fetch_xy0(GATE_ORDER[1])

            def emit_norms(PO, P):
                PB = 64
                yt_f = p_fin.tile([PB, D], bf16, tag="yt")
                yt = yt_f[:P]
                nc.sync.dma_start(yt, bounce_out[PO:PO + P, :])
                xt_f = p_fin.tile([PB, D], f32, tag="xt")
                xt = xt_f[:P]
                nc.sync.dma_start(xt, xs_f32[PO:PO + P, :])

                mu_f = p_fin.tile([PB, 1], f32, tag="mu_c")
                mu_c = mu_f[:P]
                nc.vector.tensor_reduce(mu_c, yt,
                                        mybir.AxisListType.X, Alu.add)
                nc.scalar.mul(mu_c, mu_c, 1.0 / D)
                sq_f = p_fin.tile([PB, D], f32, tag="sq_t")
                sq_t = sq_f[:P]
                ssq_f = p_fin.tile([PB, 1], f32, tag="ssq_c")
                ssq_c = ssq_f[:P]
                nc.vector.scalar_tensor_tensor(
                    sq_t, yt, 1.0, yt,
                    op0=Alu.mult, op1=Alu.mult, accum_out=ssq_c)
                nc.scalar.mul(ssq_c, ssq_c, 1.0 / D)
                musq_f = p_fin.tile([PB, 1], f32, tag="musq_c")
                musq_c = musq_f[:P]
                nc.vector.tensor_mul(musq_c, mu_c, mu_c)
                nc.vector.tensor_sub(ssq_c, ssq_c, musq_c)
                nc.vector.tensor_scalar_add(ssq_c, ssq_c, LN_EPS)
                r_f = p_fin.tile([PB, 1], f32, tag="r_c")
                r_c = r_f[:P]
                nc.vector.reciprocal(r_c, ssq_c)
                nc.scalar.sqrt(r_c, r_c)

                zt_f = p_fin.tile([PB, D], f32, tag="zt")
                zt = zt_f[:P]
                nc.vector.tensor_scalar(zt, yt, mu_c, r_c,
                                        op0=Alu.subtract, op1=Alu.mult)
                nc.vector.tensor_add(zt, zt, xt)

                rr_f = p_fin.tile([PB, 1], f32, tag="rr_c")
                rr_c = rr_f[:P]
                nc.vector.scalar_tensor_tensor(
                    sq_t, zt, 1.0, zt,
                    op0=Alu.mult, op1=Alu.mult, accum_out=rr_c)
                nc.scalar.mul(rr_c, rr_c, 1.0 / D)
                nc.vector.tensor_scalar_add(rr_c, rr_c, RMS_EPS)
                nc.vector.reciprocal(rr_c, rr_c)
                nc.scalar.sqrt(rr_c, rr_c)

                ot = sq_t
                nc.vector.tensor_scalar_mul(ot, zt, rr_c)
                nc.sync.dma_start(out_d[PO:PO + P, :], ot)

            n_rs = 0
            for seq_i, tb in enumerate(GATE_ORDER):
                xy1 = p_xy1.tile([128, NT * 512], bf16, tag="xy1")
                emit_z_gate(1, p_gate1, qsq1, ykv1, mu1, r21, wv1, tb,
                            xy1=xy1)

                subs = [c for c in RS_CHUNKS
                        if 4 * tb <= c[0] < 4 * (tb + 1)]
                for tt0, ntt in subs:
                    for tt in range(tt0, tt0 + ntt):
                        to = (tt - 4 * tb) * 128
                        for db in range(DB):
                            ps4 = p_psum.tile([128, 512], f32, tag="mm")
                            idx = 0
                            for hh in range(HPC):
                                src = xy0_t if hh == 0 else xy1
                                for nt in range(NT):
                                    nc.tensor.matmul(
                                        ps4[:],
                                        src[:, nt * 512 + to:
                                            nt * 512 + to + 128],
                                        wd_sb[:,
                                              (hh * NT + nt) * D +
                                              db * 512:
                                              (hh * NT + nt) * D +
                                              db * 512 + 512],
                                        start=(idx == 0),
                                        stop=(idx == HPC * NT - 1))
                                    idx += 1
                            ym_t = p_ym.tile([128, 512], bf16, tag="ym")
                            nc.scalar.copy(ym_t[:], ps4[:])
                            nc.sync.dma_start(
                                bounce_in[tt * 128:(tt + 1) * 128,
                                          db * 512:(db + 1) * 512],
                                ym_t[:])
                    r0 = tt0 * 128
                    rows = ntt * 128
                    o0 = r0 // NCORES
                    P = rows // NCORES
                    nc.gpsimd.collective_compute(
                        "ReduceScatter", Alu.add,
                        replica_groups=[list(range(NCORES))],
                        ins=[bounce_in[r0:r0 + rows, :].opt()],
                        outs=[bounce_out[o0:o0 + P, :].opt()])
                    n_rs += 1
                    # final norms in row batches, each emitted two RS
                    # chunks after its rows' collectives completed so
                    # the queues never head-of-line block decode work
                    if n_rs == 3:
                        emit_norms(192, 64)
                    elif n_rs == 4:
                        emit_norms(0, 64)
                    elif n_rs == 5:
                        emit_norms(64, 64)
                        emit_norms(128, 48)
                xy0_t = xy0_next
                if seq_i + 2 < len(GATE_ORDER):
                    xy0_next = fetch_xy0(GATE_ORDER[seq_i + 2])
            emit_norms(176, 16)


def build(debug=False):
    nc = bacc.Bacc("TRN2", target_bir_lowering=False, debug=False,
                   num_devices=NCORES)
    f32, bf16 = dt.float32, dt.bfloat16
    tens = {
        'x_bf': nc.dram_tensor("x_bf", [T, D], bf16, kind="ExternalInput"),
        'xT_bf': nc.dram_tensor("xT_bf", [D, T], bf16, kind="ExternalInput"),
        'xs_f32': nc.dram_tensor("xs_f32", [TS, D], f32,
                                 kind="ExternalInput"),
        'wencT': nc.dram_tensor("wencT", [HPC, D, N], bf16,
                                kind="ExternalInput"),
        'wencvT': nc.dram_tensor("wencvT", [HPC, D, N], bf16,
                                 kind="ExternalInput"),
        'wdecT': nc.dram_tensor("wdecT", [HPC * N, D], bf16,
                                kind="ExternalInput"),
        'wsumT': nc.dram_tensor("wsumT", [HPC, 128, NT], f32,
                                kind="ExternalInput"),
        'cosT': nc.dram_tensor("cosT", [128, T], bf16, kind="ExternalInput"),
        'sinT': nc.dram_tensor("sinT", [128, T], bf16, kind="ExternalInput"),
        'masks': nc.dram_tensor("masks", [4, 128, 512], bf16,
                                kind="ExternalInput"),
        'out': nc.dram_tensor("out", [TS, D], f32, kind="ExternalOutput"),
        'xy_d': nc.dram_tensor("xy_d", [NT, 128, T], bf16, kind="Internal"),
        'bounce_in': nc.dram_tensor("bounce_in", [T, D], bf16,
                                    kind="Internal"),
        'bounce_out': nc.dram_tensor("bounce_out", [TS, D], bf16,
                                     kind="Internal"),
        'warm_in': nc.dram_tensor("warm_in", [128, 64], bf16,
                                  kind="Internal"),
        'warm_out': nc.dram_tensor("warm_out", [16, 64], bf16,
                                   kind="Internal"),
    }

    with tile.TileContext(nc) as tc:
        _emit(nc, tc, tens)
    nc.compile()
    return nc


def xs_k(x2, k):
    # xs_f32 row o0+i must match bounce_out row o0+i (o0 = tt0*128/8)
    xs = np.empty((TS, D), np.float32)
    for tt0, ntt in RS_CHUNKS:
        pp = ntt * 128 // NCORES
        o0 = tt0 * 128 // NCORES
        g = tt0 * 128 + pp * k
        xs[o0:o0 + pp] = x2[g:g + pp]
    return np.ascontiguousarray(xs)


def make_in_maps(x, W_enc, W_enc_v, W_dec):
    x2 = np.asarray(x, FP32).reshape(T, D)
    x_bf = x2.astype(BF)
    xT_bf = np.ascontiguousarray(x2.T).astype(BF)
    cosT, sinT = _rope_tables()
    cosT, sinT = cosT.astype(BF), sinT.astype(BF)
    masks = _masks().astype(BF)
    wsum = np.asarray(W_enc_v, FP32).sum(axis=2)          # (NH, N)

    in_maps = []
    for k in range(NCORES):
        h0 = HPC * k
        wencT = np.ascontiguousarray(
            np.asarray(W_enc[h0:h0 + HPC], FP32).transpose(0, 2, 1)
        ).astype(BF)
        wencvT = np.ascontiguousarray(
            np.asarray(W_enc_v[h0:h0 + HPC], FP32).transpose(0, 2, 1)
        ).astype(BF)
        wdecT = np.ascontiguousarray(
            np.asarray(W_dec[:, h0 * N:(h0 + HPC) * N], FP32).T
        ).astype(BF)
        wsumT = np.ascontiguousarray(
            wsum[h0:h0 + HPC].reshape(HPC, NT, 128).transpose(0, 2, 1))
        in_maps.append({
            'x_bf': x_bf,
            'xT_bf': xT_bf,
            'xs_f32': xs_k(x2, k),
            'wencT': wencT,
            'wencvT': wencvT,
            'wdecT': wdecT,
            'wsumT': wsumT,
            'cosT': cosT,
            'sinT': sinT,
            'masks': masks,
        })
    return in_maps


_nc_cache = {}


def get_nc(debug=False):
    if debug not in _nc_cache:
        _nc_cache[debug] = build(debug=debug)
    return _nc_cache[debug]


def run(x, W_enc, W_enc_v, W_dec, debug=False, trace=False):
    nc = get_nc(debug=debug)
    in_maps = make_in_maps(x, W_enc, W_enc_v, W_dec)
    res = bass_utils.run_bass_kernel_spmd(
        nc, in_maps, core_ids=list(range(NCORES)), trace=trace)
    # chunked reduce-scatter: core c's piece i holds the c-th 1/8 of
    # chunk i's row range
    out = np.empty((T, D), np.float32)
    for c in range(NCORES):
        oc = res.results[c]['out']
        for tt0, ntt in RS_CHUNKS:
            pp = ntt * 128 // NCORES
            o0 = tt0 * 128 // NCORES
            g = tt0 * 128 + pp * c
            out[g:g + pp] = oc[o0:o0 + pp]
    return out.reshape(B, T, D), res


def kernel(x, W_enc, W_enc_v, W_dec):
    out, _ = run(x, W_enc, W_enc_v, W_dec)
    return out.astype(np.float32)
